# revision 26
# baseline (speedup 1.0000x reference)
"""Self-contained Trainium2 kernel for the moe_routing tree-walk problem.

Problem (hardcoded shapes): x [16384, 1024] f32, keys/values [4095, 8, 1024]
f32. For each sample and each of 8 trees, walk a depth-12 binary tree: at
each level lam = <x, key[node]>, y += lam * value[node],
node = 2*node + 1 + (lam > 0).

Strategy: data-parallel over the batch across 8 NeuronCores (2048 samples
per core), tables replicated per core.

Per 128-sample tile:
- Levels 0..7 ("dense"): lam for ALL nodes of the level is computed on the
  tensor engine as xT-chunk @ transposed-key-slab matmuls (slabs are
  SBUF-resident); lam is evacuated PSUM->SBUF on ACT (so PSUM banks recycle
  without waiting on DVE), then the per-sample lam is selected with an
  iota/is_equal mask and a fused multiply+row-reduce. The y update uses a
  one-hot-times-lam matrix W^T (built from PE-transposed node/lam rows via
  a tiny bf16 expansion matmul, so it lands nodes-on-partitions) and
  accumulates W^T.T @ V_slab into PSUM-resident y. No gathers here.
- Levels 8..10 ("deep"): per-(sample,tree) [f32 key | bf16 value] rows
  fetched with single-index indirect gather DMAs; lam via fused
  multiply+row-reduce on DVE; y += diag(lam) @ v_row on the tensor engine.
  Tree state is kept in two 4-tree halves so one half's next-level gathers
  launch while the other half is still in flight.
- Level 11: lam's sign is never used for routing (no level 12), so this
  level gathers from a separate all-bf16 [key | value] table — 4KB rows
  instead of 6KB, cutting gather traffic by a third for that level.

Emission is software-pipelined: the dense phase of tile t+1 is emitted
interleaved with the deep levels of tile t, so the Tile scheduler bakes an
engine order where dense compute hides under gather DMA (DMA engines are
>93% occupied in the cost-model timeline; the kernel is at the HBM traffic
roofline for its ~430MB/core of gathers + slabs).

Values are cast to bf16 on the host (halves value traffic; y error ~1e-3
relative). Keys/x/lam stay f32 for levels 0..10 so routing matches the
reference up to f32 rounding (a route flip near lam==0 would swap whole
subtree contributions, so sign precision is critical everywhere except the
last level).
"""

import numpy as np
import ml_dtypes

from contextlib import ExitStack

import concourse.bass as bass
import concourse.mybir as mybir
import concourse.tile as tile
from concourse.bass import IndirectOffsetOnAxis
from concourse.bass_utils import run_bass_kernel_spmd
from concourse.masks import make_identity

# ---------------------------------------------------------------------------
# Workaround: this walrus build rejects >1 sync wait on most instructions
# ("Too many sync wait commands"), but the Tile scheduler can attach several.
# Post-pass: move excess waits onto single-wait NoOps inserted just before
# the instruction on the same engine (program order makes this equivalent).
_WSPLIT_COUNT = [0]


def _split_multi_waits(nc, cap=1):
    for fn in nc.m.functions:
        for bb in fn.blocks:
            out = []
            changed = False
            for inst in list(bb.instructions):
                si = inst.sync_info
                if si is not None and si.on_wait and len(si.on_wait) > cap:
                    waits = list(si.on_wait)
                    extra, keep = waits[:-cap], waits[-cap:]
                    for w in extra:
                        _WSPLIT_COUNT[0] += 1
                        nop = mybir.InstNoOp(
                            name=f"WSPLIT-{_WSPLIT_COUNT[0]}", ins=[], outs=[]
                        )
                        nop.engine = inst.engine
                        nop.sync_info = mybir.SyncInfo(on_wait=[w], on_update=[])
                        out.append(nop)
                    inst.sync_info = mybir.SyncInfo(
                        on_wait=keep, on_update=list(si.on_update or [])
                    )
                    changed = True
                out.append(inst)
            if changed:
                bb.instructions = out
# ---------------------------------------------------------------------------

F32 = mybir.dt.float32
BF16 = mybir.dt.bfloat16
I32 = mybir.dt.int32
OP = mybir.AluOpType

N_CORES = 8
B, D, K, DEPTH = 16384, 1024, 8, 12
N_NODES = 2 ** DEPTH - 1  # 4095
BL = B // N_CORES         # 2048 samples per core
DENSE_LEVELS = 8
P = 128
KH = K // 2               # deep-phase half width (4 trees)


def _pad_layout(dense_levels):
    pad_offs, off = [], 0
    for l in range(dense_levels):
        pad_offs.append(off)
        off += ((K * 2 ** l + P - 1) // P) * P
    return pad_offs, off


def host_prep(x_shard, keys, values, dense_levels=DENSE_LEVELS):
    """keys/values: [n_nodes, K, D] f32 arrays. Returns per-core input dict
    pieces shared across cores (slabs) and the x-derived arrays."""
    Bl, Dd = x_shard.shape
    T = Bl // P
    DC = Dd // P
    L = dense_levels

    xT4 = np.ascontiguousarray(
        x_shard.reshape(T, P, DC, P).transpose(3, 0, 2, 1)
    )

    ksecs = []
    for l in range(L):
        base, N_l = 2 ** l - 1, 2 ** l
        kl = keys[base:base + N_l]
        klT = kl.transpose(2, 1, 0).reshape(DC, P, K * N_l)
        ksecs.append(klT)
    kTs = np.ascontiguousarray(np.concatenate(ksecs, axis=2).transpose(1, 0, 2))

    vsecs, esecs = [], []
    pad_offs, CSHP = _pad_layout(L)
    for l in range(L):
        base, N_l = 2 ** l - 1, 2 ** l
        rows = K * N_l
        prows = ((rows + P - 1) // P) * P
        vl = np.zeros((prows, Dd), dtype=np.float32)
        vl[:rows] = values[base:base + N_l].transpose(1, 0, 2).reshape(rows, Dd)
        vsecs.append(vl)
        el = np.zeros((9, prows), dtype=np.float32)
        for t in range(K):
            el[t, t * N_l:(t + 1) * N_l] = 1.0
        el[8, :rows] = -np.tile(np.arange(N_l, dtype=np.float32), K)
        el[8, rows:] = 1e9
        esecs.append(el)
    vsh = np.ascontiguousarray(np.concatenate(vsecs, axis=0)).astype(
        ml_dtypes.bfloat16)
    # bf16 is exact for the small ints the expand matmuls produce (node ids
    # <= 127, tree masks); lam only weights y so bf16 is fine there too.
    expand = np.ascontiguousarray(np.concatenate(esecs, axis=1)).astype(
        ml_dtypes.bfloat16)
    return xT4, kTs, vsh, expand, CSHP


def build_kernel(nc, *, Bl=BL, Dd=D, depth=DEPTH, n_nodes=N_NODES,
                 dense_levels=DENSE_LEVELS, repeat=1):
    NK = n_nodes * K
    T = Bl // P
    DC = Dd // P
    L = dense_levels
    CK = K * (2 ** L - 1)
    pad_offs, CSHP = _pad_layout(L)
    NCH = CSHP // P

    koffs = [K * (2 ** l - 1) for l in range(L + 1)]

    x_d = nc.declare_dram_parameter("x", [Bl, Dd], F32, isOutput=False)
    xT_d = nc.declare_dram_parameter("xT4", [P, T, DC, P], F32, isOutput=False)
    kTs_d = nc.declare_dram_parameter("kTs", [P, DC, CK], F32, isOutput=False)
    vsh_d = nc.declare_dram_parameter("vsh", [CSHP, Dd], BF16, isOutput=False)
    exp_d = nc.declare_dram_parameter("expand", [9, CSHP], BF16, isOutput=False)
    KVW = Dd + Dd // 2   # f32 key row + bf16 value row viewed as f32
    kv_d = nc.declare_dram_parameter("kv", [NK, KVW], F32, isOutput=False)
    # Last level: lam's sign is never used for routing, so bf16 keys suffice
    # (lam only weights y). Packed [key bf16 | value bf16] rows, level-local.
    N11 = 2 ** (depth - 1)
    kv11_d = nc.declare_dram_parameter("kv11", [N11 * K, 2 * Dd], BF16,
                                       isOutput=False)
    y_d = nc.declare_dram_parameter("y", [Bl, Dd], F32, isOutput=True)

    n_half = (Dd + 511) // 512
    NH = Dd // n_half
    max_Nl = 2 ** (L - 1)

    with ExitStack() as ctx:
        tc = ctx.enter_context(tile.TileContext(nc))
        const_p = ctx.enter_context(tc.tile_pool(name="const", bufs=1))
        xp = ctx.enter_context(tc.tile_pool(name="x", bufs=3))
        gp = ctx.enter_context(tc.tile_pool(name="gather", bufs=5))
        sp = ctx.enter_context(tc.tile_pool(name="small", bufs=8))
        scr = ctx.enter_context(tc.tile_pool(name="scratch", bufs=3))
        lsb = ctx.enter_context(tc.tile_pool(name="lamsb", bufs=3))
        wp = ctx.enter_context(tc.tile_pool(name="wtile", bufs=4))
        yp = ctx.enter_context(tc.tile_pool(name="yout", bufs=2))
        psy = ctx.enter_context(tc.tile_pool(name="psy", bufs=2, space="PSUM"))
        psl = ctx.enter_context(tc.tile_pool(name="psl", bufs=2, space="PSUM"))
        psb = ctx.enter_context(tc.tile_pool(name="psb", bufs=2, space="PSUM"))

        exp_sb = const_p.tile([9, CSHP], BF16)
        nc.sync.dma_start(exp_sb[:], exp_d[:])
        # Per-level slab loads so tile 0's level l only waits for its slice.
        kTs_sb = const_p.tile([P, DC, CK], F32)
        for l in range(L):
            nc.sync.dma_start(kTs_sb[:, :, koffs[l]:koffs[l + 1]],
                              kTs_d[:, :, koffs[l]:koffs[l + 1]])
        vsh_sb = const_p.tile([P, NCH, Dd], BF16)
        for l in range(L):
            q0 = pad_offs[l] // P
            q1 = (pad_offs[l] + ((K * 2 ** l + P - 1) // P) * P) // P
            nc.sync.dma_start(
                vsh_sb[:, q0:q1, :],
                vsh_d[q0 * P:q1 * P, :].rearrange("(q p) d -> p q d", p=P))

        ident_b = const_p.tile([P, P], BF16)
        make_identity(nc, ident_b[:])
        ident_f = const_p.tile([P, P], F32)
        make_identity(nc, ident_f[:])
        tree_off = const_p.tile([P, K], I32)
        nc.gpsimd.iota(tree_off[:], pattern=[[1, K]], base=0,
                       channel_multiplier=0)
        iota_row_i = const_p.tile([P, max_Nl], I32)
        nc.gpsimd.iota(iota_row_i[:], pattern=[[1, max_Nl]], base=0,
                       channel_multiplier=0)
        iota_row = const_p.tile([P, max_Nl], F32)
        nc.vector.tensor_copy(iota_row[:], iota_row_i[:])
        rowsN = const_p.tile([9, P], BF16)
        nc.vector.memset(rowsN[:], 1.0)   # row 8 stays 1.0 (ones row)
        rowsL = const_p.tile([9, P], BF16)
        nc.vector.memset(rowsL[:], 0.0)   # row 8 stays 0.0

        class TileState:
            pass

        def start_tile(rep, t):
            st = TileState()
            st.rep, st.t = rep, t
            st.x_tile = xp.tile([P, Dd], F32, tag="xt")
            nc.sync.dma_start(st.x_tile[:], x_d[t * P:(t + 1) * P, :])
            st.xT_tile = xp.tile([P, DC, P], F32, tag="xT")
            nc.sync.dma_start(st.xT_tile[:], xT_d[:, t, :, :])
            st.node = sp.tile([P, K], F32, tag="node")
            nc.vector.memset(st.node[:], 0.0)
            st.lam = sp.tile([P, K], F32, tag="lam")
            st.y_ps = [
                psy.tile([P, NH], F32, tag=f"yps{h}",
                         name=f"yps{h}_{rep}_{t}")
                for h in range(n_half)
            ]
            st.mm_first = True
            return st

        def y_accum(st, lhsT, rhs_full, last):
            for h in range(n_half):
                nc.tensor.matmul(
                    out=st.y_ps[h][:],
                    lhsT=lhsT,
                    rhs=rhs_full[:, h * NH:(h + 1) * NH],
                    start=st.mm_first,
                    stop=last,
                )
            st.mm_first = False

        def dense_level(st, l):
            rep, t = st.rep, st.t
            N_l = 2 ** l
            CL = K * N_l
            junk2 = scr.tile([P, max_Nl], F32, tag="junk2")
            for blk in range(0, CL, 512):
                bw = min(512, CL - blk)
                lam_ps = psl.tile([P, bw], F32, tag="lamall",
                                  name=f"lamall_{rep}_{t}_{l}_{blk}")
                for c in range(DC):
                    nc.tensor.matmul(
                        out=lam_ps[:],
                        lhsT=st.xT_tile[:, c, :],
                        rhs=kTs_sb[:, c,
                                   koffs[l] + blk:koffs[l] + blk + bw],
                        start=(c == 0),
                        stop=(c == DC - 1),
                    )
                # Evacuate PSUM promptly (ACT) so the bank recycles without
                # waiting on DVE's select backlog.
                lam_sb = lsb.tile([P, 512], F32, tag="lamsb")
                nc.scalar.activation(lam_sb[:, :bw], lam_ps[:],
                                     mybir.ActivationFunctionType.Copy)
                for k in range(blk // N_l, (blk + bw) // N_l):
                    nc.vector.scalar_tensor_tensor(
                        out=junk2[:, :N_l],
                        in0=iota_row[:, :N_l],
                        scalar=st.node[:, k:k + 1],
                        in1=lam_sb[:, k * N_l - blk:(k + 1) * N_l - blk],
                        op0=OP.is_equal,
                        op1=OP.mult,
                        accum_out=st.lam[:, k:k + 1],
                    )

            tr_ps = psb.tile([P, 512], F32, tag="trbc",
                             name=f"trbc_{rep}_{t}_{l}")
            nc.tensor.transpose(tr_ps[0:K, 256:256 + P], st.node[:],
                                ident_f[:])
            nc.tensor.transpose(tr_ps[0:K, 256 + P:256 + 2 * P],
                                st.lam[:], ident_f[:])
            nc.scalar.activation(
                rowsN[0:K, :], tr_ps[0:K, 256:256 + P],
                mybir.ActivationFunctionType.Copy)
            nc.scalar.activation(
                rowsL[0:K, :], tr_ps[0:K, 256 + P:256 + 2 * P],
                mybir.ActivationFunctionType.Copy)

            n_chunks = (K * N_l + P - 1) // P
            for q in range(n_chunks):
                ecols = exp_sb[:, pad_offs[l] + q * P:
                               pad_offs[l] + (q + 1) * P]
                bc_ps = psb.tile([P, 512], F32, tag="trbc",
                                 name=f"bcps_{rep}_{t}_{l}_{q}")
                nc.tensor.matmul(out=bc_ps[:, 0:P], lhsT=ecols,
                                 rhs=rowsN[:], start=True, stop=True)
                nc.tensor.matmul(out=bc_ps[:, P:256], lhsT=ecols,
                                 rhs=rowsL[:], start=True, stop=True)
                tmp = scr.tile([P, P], F32, tag="wtmp")
                nc.vector.tensor_scalar(
                    out=tmp[:], in0=bc_ps[:, 0:P], scalar1=0.0,
                    scalar2=None, op0=OP.is_equal,
                )
                W = wp.tile([P, P], BF16, tag="W")
                nc.vector.tensor_tensor(
                    out=W[:], in0=tmp[:], in1=bc_ps[:, P:256],
                    op=OP.mult,
                )
                y_accum(st, W[:], vsh_sb[:, pad_offs[l] // P + q, :], False)

            # node = node*2 + (lam > 0)   (relative index, f32 exact)
            gt = sp.tile([P, K], F32, tag="gt")
            nc.vector.tensor_scalar(
                out=gt[:], in0=st.lam[:], scalar1=0.0, scalar2=None,
                op0=OP.is_gt,
            )
            nc.vector.tensor_scalar(
                out=st.node[:], in0=st.node[:], scalar1=2.0, scalar2=0.0,
                op0=OP.mult, op1=OP.add,
            )
            nc.vector.tensor_tensor(
                out=st.node[:], in0=st.node[:], in1=gt[:], op=OP.add
            )

        def deep_entry(st):
            # Split tree state into two 4-tree halves so each half's next
            # level can launch without waiting for the other half's lams.
            st.nodeh = []
            for h in range(2):
                nh = sp.tile([P, KH], F32, tag=f"nodeh{h}")
                nc.vector.tensor_copy(nh[:], st.node[:, h * KH:(h + 1) * KH])
                st.nodeh.append(nh)

        def deep_half_idx(st, h, l):
            """idx for half h at deep level l (kv row = node*K + base*K + k).
            The last level's table (kv11) is level-local, so base = 0."""
            base = 2 ** l - 1 if l < depth - 1 else 0
            nodeg = sp.tile([P, KH], F32, tag=f"nodeg{h}")
            nc.vector.tensor_scalar(
                out=nodeg[:], in0=st.nodeh[h][:], scalar1=float(K),
                scalar2=float(base * K), op0=OP.mult, op1=OP.add,
            )
            idx = sp.tile([P, KH], I32, tag=f"idx{h}")
            nc.vector.tensor_copy(idx[:], nodeg[:])
            nc.vector.tensor_tensor(
                out=idx[:], in0=idx[:], in1=tree_off[:, h * KH:(h + 1) * KH],
                op=OP.add
            )
            return idx

        def deep_half(st, h, l, idx, last_lvl):
            """Gather + lam + y for trees of half h at level l; update node."""
            rep, t = st.rep, st.t
            lamh = sp.tile([P, KH], F32, tag=f"lamh{h}")
            for j in range(KH):
                k = h * KH + j
                if last_lvl:
                    kvg = gp.tile([P, 2 * Dd], BF16, tag="kvg")
                    nc.gpsimd.indirect_dma_start(
                        out=kvg[:],
                        out_offset=None,
                        in_=kv11_d[:],
                        in_offset=IndirectOffsetOnAxis(ap=idx[:, j:j + 1],
                                                       axis=0),
                    )
                    key_ap, vg = kvg[:, 0:Dd], kvg[:, Dd:2 * Dd]
                else:
                    kvg = gp.tile([P, KVW], F32, tag="kvg")
                    nc.gpsimd.indirect_dma_start(
                        out=kvg[:],
                        out_offset=None,
                        in_=kv_d[:],
                        in_offset=IndirectOffsetOnAxis(ap=idx[:, j:j + 1],
                                                       axis=0),
                    )
                    key_ap, vg = kvg[:, 0:Dd], kvg[:, Dd:KVW].bitcast(BF16)
                junk = scr.tile([P, Dd], F32, tag="junk")
                nc.vector.scalar_tensor_tensor(
                    out=junk[:],
                    in0=st.x_tile[:],
                    scalar=1.0,
                    in1=key_ap,
                    op0=OP.mult,
                    op1=OP.mult,
                    accum_out=lamh[:, j:j + 1],
                )
                diag = wp.tile([P, P], BF16, tag="diag")
                nc.scalar.activation(
                    diag[:], ident_b[:],
                    mybir.ActivationFunctionType.Copy,
                    scale=lamh[:, j:j + 1],
                )
                y_accum(st, diag[:], vg, last_lvl and k == K - 1)
            if not last_lvl:
                # nodeh = nodeh*2 + (lamh > 0)
                gth = sp.tile([P, KH], F32, tag=f"gth{h}")
                nc.vector.tensor_scalar(
                    out=gth[:], in0=lamh[:], scalar1=0.0, scalar2=None,
                    op0=OP.is_gt,
                )
                nc.vector.tensor_scalar(
                    out=st.nodeh[h][:], in0=st.nodeh[h][:], scalar1=2.0,
                    scalar2=0.0, op0=OP.mult, op1=OP.add,
                )
                nc.vector.tensor_tensor(
                    out=st.nodeh[h][:], in0=st.nodeh[h][:], in1=gth[:],
                    op=OP.add
                )

        def deep_level(st, l):
            last_lvl = (l == depth - 1)
            for h in range(2):
                idx = deep_half_idx(st, h, l)
                deep_half(st, h, l, idx, last_lvl)

        def finish_tile(st):
            t = st.t
            y_sb = yp.tile([P, Dd], F32)
            for h in range(n_half):
                nc.scalar.activation(
                    y_sb[:, h * NH:(h + 1) * NH], st.y_ps[h][:],
                    mybir.ActivationFunctionType.Copy)
            nc.sync.dma_start(y_d[t * P:(t + 1) * P, :], y_sb[:])

        # Software-pipelined emission: dense chunks of tile t+1 are emitted
        # between the deep levels of tile t so the scheduler bakes an engine
        # order where dense compute hides under gather DMA.
        DENSE_CHUNKS = [list(range(0, L - 2)), [L - 2], [L - 1]]

        def emit_dense_chunk(st, ci):
            for l in DENSE_CHUNKS[ci]:
                dense_level(st, l)
            if ci == len(DENSE_CHUNKS) - 1:
                deep_entry(st)

        for rep in range(repeat):
            cur = start_tile(rep, 0)
            for ci in range(len(DENSE_CHUNKS)):
                emit_dense_chunk(cur, ci)
            for t in range(T):
                nxt = None
                if t + 1 < T:
                    nxt = start_tile(rep, t + 1)
                for li, l in enumerate(range(L, depth)):
                    deep_level(cur, l)
                    if nxt is not None and li < len(DENSE_CHUNKS):
                        emit_dense_chunk(nxt, li)
                finish_tile(cur)
                cur = nxt

    return nc


_NC_CACHE = {}


def _get_nc(repeat=1):
    key = ("nc", repeat)
    if key not in _NC_CACHE:
        nc = bass.Bass("TRN2", target_bir_lowering=False, debug=False,
                       num_devices=N_CORES)
        build_kernel(nc, repeat=repeat)
        _split_multi_waits(nc)
        _NC_CACHE[key] = nc
    return _NC_CACHE[key]


def make_kv(keys_flat_f32, values_flat_bf16):
    NK, Dd = keys_flat_f32.shape
    kv = np.empty((NK, Dd + Dd // 2), dtype=np.float32)
    kv[:, :Dd] = keys_flat_f32
    kv[:, Dd:] = values_flat_bf16.view(np.float32)
    return kv


def _prep_inputs(x, keys, values):
    x = np.ascontiguousarray(np.asarray(x, dtype=np.float32))
    keys = np.asarray(keys, dtype=np.float32)
    values = np.asarray(values, dtype=np.float32)
    keys_flat = np.ascontiguousarray(keys.reshape(N_NODES * K, D))
    values_flat = np.ascontiguousarray(values.reshape(N_NODES * K, D)).astype(
        ml_dtypes.bfloat16)
    kv = make_kv(keys_flat, values_flat)

    # Level-11 table: bf16 keys + bf16 values, level-local rows (n_rel*K + k)
    base11 = 2 ** (DEPTH - 1) - 1  # 2047
    n11 = 2 ** (DEPTH - 1)         # 2048
    kv11 = np.empty((n11 * K, 2 * D), dtype=ml_dtypes.bfloat16)
    kv11[:, :D] = keys[base11:base11 + n11].reshape(n11 * K, D).astype(
        ml_dtypes.bfloat16)
    kv11[:, D:] = values[base11:base11 + n11].reshape(n11 * K, D).astype(
        ml_dtypes.bfloat16)
    kv11 = np.ascontiguousarray(kv11)

    # table-derived slabs are identical for every core: compute them once
    _, kTs, vsh, expand, _ = host_prep(x[:BL], keys, values)
    in_maps = []
    for c in range(N_CORES):
        x_shard = x[c * BL:(c + 1) * BL]
        T = BL // P
        DC = D // P
        xT4 = np.ascontiguousarray(
            x_shard.reshape(T, P, DC, P).transpose(3, 0, 2, 1))
        in_maps.append({
            "x": x_shard,
            "xT4": xT4,
            "kTs": kTs,
            "vsh": vsh,
            "expand": expand,
            "kv": kv,
            "kv11": kv11,
        })
    return in_maps


def kernel(x, keys, values):
    nc = _get_nc()
    in_maps = _prep_inputs(x, keys, values)
    res = run_bass_kernel_spmd(nc, in_maps, list(range(N_CORES)))
    y = np.concatenate([res.results[c]["y"] for c in range(N_CORES)], axis=0)
    return y.astype(np.float32)
